# revision 1
# baseline (speedup 1.0000x reference)
"""Trainium2 Bass kernel for nn_DecoderLayer_19851338842283.

Strategy (8 NeuronCores): data-parallel over batch (4) x tensor-parallel (2)
over heads (8 each) + mlp_dim (2048 each).  Each core computes partial
attention + MLP outputs for one batch; the host sums the two tensor-parallel
partials and adds the residual.  No on-device collectives.

Device-side layout is fully transpose-free:
  - host passes x^T [E, L], so projections produce q^T/k^T in [head*d, L]
    layout and v in natural [L, head*d] layout directly.
  - scores are computed transposed ([k, q]); softmax denominators come from a
    fused ones-column in the v operand (M=65 matmuls); the T5 relative-
    position bias + causal mask is a host-precomputed Toeplitz band [128,640]
    per head (exact for |q-k| <= 511; bias is constant for q-k >= 113, which
    cancels in softmax) added on VectorE before the exponent.
  - all matmuls run as float32r (TF32-like, full PE rate at N>=512).
"""

import ml_dtypes
import numpy as np

import concourse.bacc as bacc
import concourse.mybir as mybir
import concourse.tile as tile
from concourse.bass_utils import run_bass_kernel_spmd

F32 = mybir.dt.float32
F32R = mybir.dt.float32r
BF16 = mybir.dt.bfloat16
Act = mybir.ActivationFunctionType
Alu = mybir.AluOpType

B, L, E, H, D, F = 4, 2048, 1024, 16, 64, 4096
HC = H // 2          # heads per core = 8
FC = F // 2          # mlp dim per core = 2048
NCORES = 8
ET = E // 128        # 8  e-tiles
LT = L // 128        # 16 l-tiles
PT = HC * D // 128   # 4  head-pair tiles
FT = FC // 128       # 16 f-tiles
NSUP = L // 512      # 4  q-supers
USE_BF16_MLP2 = False
BAND_OFF = 128
BAND_W = 384
NUM_BUCKETS = 32


MM_LOG = []


def _build(causal: bool):
    MM_LOG.clear()
    nc = bacc.Bacc("TRN2", target_bir_lowering=False, debug=False,
                   num_devices=NCORES)
    _real_mm = nc.tensor.matmul

    def _mm(*a, **k):
        import sys as _s
        MM_LOG.append(f"{_s._getframe(1).f_lineno}")
        return _real_mm(*a, **k)

    nc.tensor.matmul = _mm
    xT_d = nc.dram_tensor("xT", [E, L], F32R, kind="ExternalInput").ap()
    wq_d = nc.dram_tensor("wq", [E, HC * D], F32R, kind="ExternalInput").ap()
    wk_d = nc.dram_tensor("wk", [E, HC * D], F32R, kind="ExternalInput").ap()
    wv_d = nc.dram_tensor("wv", [E, HC * D], F32R, kind="ExternalInput").ap()
    wo_d = nc.dram_tensor("wo", [HC * D, E], F32R, kind="ExternalInput").ap()
    wi_d = nc.dram_tensor("wi", [E, FC], F32R, kind="ExternalInput").ap()
    wmo_d = nc.dram_tensor("wmo", [FC, E],
                           BF16 if USE_BF16_MLP2 else F32R,
                           kind="ExternalInput").ap()
    band_d = nc.dram_tensor("band", [128, HC, BAND_W], F32R,
                            kind="ExternalInput").ap()
    ident_d = nc.dram_tensor("ident", [128, 128], F32R,
                             kind="ExternalInput").ap()
    bfut_d = nc.dram_tensor("bfut", [128, HC], F32, kind="ExternalInput").ap()
    attn_d = nc.dram_tensor("attn_out", [L, E], F32, kind="ExternalOutput").ap()
    mlp_d = nc.dram_tensor("mlp_out", [L, E], F32, kind="ExternalOutput").ap()
    mdt = BF16 if USE_BF16_MLP2 else F32R
    hT_d = nc.dram_tensor("hT_scr", [FT, 128, L], mdt).ap()

    with tile.TileContext(nc) as tc:
        with (
            tc.tile_pool(name="pps", bufs=4, space="PSUM") as pps,
            tc.tile_pool(name="pctx", bufs=4, space="PSUM") as pctx,
        ):
            with (
                tc.tile_pool(name="pct", bufs=6) as pct,
                tc.tile_pool(name="pwo", bufs=1) as pwo,
            ):

                def outproj(s, cts, pob, chunks, use_act=False):
                    qs = 512 * s
                    for qt, ec in chunks:
                        acc = (pps if use_act else pctx).tile(
                            [128, 512], F32,
                            tag="ps" if use_act else "ctx", name="opacc")
                        for p in range(PT):
                            nc.tensor.matmul(
                                acc[:],
                                cts[p][:, 128 * qt:128 * qt + 128],
                                wos[:, p, 512 * ec:512 * ec + 512],
                                start=(p == 0), stop=(p == PT - 1))
                        ob = pob.tile([128, 512], F32, tag="ob",
                                      name="opob")
                        if use_act:
                            nc.scalar.copy(ob[:], acc[:])
                        else:
                            nc.vector.tensor_copy(ob[:], acc[:])
                        nc.sync.dma_start(
                            attn_d[qs + 128 * qt:qs + 128 * qt + 128,
                                   512 * ec:512 * ec + 512], ob[:])

                with (
                    tc.tile_pool(name="pqk", bufs=2 * PT) as pqk,
                    tc.tile_pool(name="pva", bufs=1) as pva,
                ):
                    qT = [pqk.tile([128, L], F32R, tag="qk", name=f"qT{i}") for i in range(PT)]
                    kT = [pqk.tile([128, L], F32R, tag="qk", name=f"kT{i}") for i in range(PT)]
                    va_all = pva.tile([128, LT * HC * 65], F32R, tag="va",
                                      name="va_all")

                    # ---------------- phase 1: projections + MLP-in ----------
                    with tc.tile_pool(name="pxT", bufs=ET) as pxT:
                        xt = [pxT.tile([128, L], F32R, tag="xT", name=f"xt{i}")
                              for i in range(ET)]

                        # q and k projections (q scaled by 1/sqrt(D)=1/8)
                        with tc.tile_pool(name="pw", bufs=4) as pw:
                            wq3h = wq_d.rearrange("(et ep) hd -> ep et hd",
                                                  ep=128)
                            ws0a = pw.tile([128, ET // 2, 128], F32R,
                                           tag="w", name="ws0a")
                            nc.sync.dma_start(ws0a[:],
                                              wq3h[:, 0:ET // 2, 0:128])
                            ws0b = pw.tile([128, ET // 2, 128], F32R,
                                           tag="w", name="ws0b")
                            nc.sync.dma_start(ws0b[:],
                                              wq3h[:, ET // 2:ET, 0:128])
                            for c in range(NSUP):
                                for t in range(ET):
                                    nc.sync.dma_start(
                                        xt[t][:, 512 * c:512 * c + 512],
                                        xT_d[128 * t:128 * t + 128,
                                             512 * c:512 * c + 512])
                            for w_d, dst, scale in ((wq_d, qT, 0.125),
                                                    (wk_d, kT, None)):
                                w3 = w_d.rearrange("(et ep) hd -> ep et hd", ep=128)
                                for p in range(PT):
                                    if scale is not None and p == 0:
                                        wsa, wsb = ws0a, ws0b
                                    else:
                                        wsa = pw.tile([128, ET // 2, 128],
                                                      F32R, tag="w",
                                                      name="wsa")
                                        nc.sync.dma_start(
                                            wsa[:],
                                            w3[:, 0:ET // 2,
                                               128 * p:128 * p + 128])
                                        wsb = pw.tile([128, ET // 2, 128],
                                                      F32R, tag="w",
                                                      name="wsb")
                                        nc.sync.dma_start(
                                            wsb[:],
                                            w3[:, ET // 2:ET,
                                               128 * p:128 * p + 128])
                                    for c in range(NSUP):
                                        acc = pps.tile([128, 512], F32, tag="ps")
                                        for t in range(ET):
                                            wh = wsa if t < ET // 2 else wsb
                                            nc.tensor.matmul(
                                                acc[:],
                                                wh[:, t % (ET // 2), :],
                                                xt[t][:, 512 * c:512 * c + 512],
                                                start=(t == 0), stop=(t == ET - 1))
                                        o = dst[p][:, 512 * c:512 * c + 512]
                                        if scale is None:
                                            nc.scalar.copy(o, acc[:])
                                        else:
                                            nc.scalar.mul(o, acc[:], scale)

                        # v projection into [L, hd] with interleaved ones columns
                        with tc.tile_pool(name="pwv", bufs=2) as pwv:
                            wv3 = wv_d.rearrange("(et ep) hd -> ep et hd", ep=128)
                            ones_c = nc.const_aps.tensor(1.0, [128, HC, 1], F32)
                            wvss = []
                            for vh in range(2):
                                wvs = pwv.tile([128, ET, 256], F32R, tag="wv",
                                               name=f"wvs{vh}")
                                wvss.append(wvs)
                                for t in range(ET):
                                    nc.sync.dma_start(
                                        wvs[:, t, :],
                                        wv3[:, t, 256 * vh:256 * vh + 256])
                            for vh in range(2):   # hd halves = head quads
                                wvs = wvss[vh]
                                for lt in range(LT):
                                    acc = pps.tile([128, 512], F32, tag="ps")
                                    for t in range(ET):
                                        nc.tensor.matmul(
                                            acc[:, 0:256],
                                            xt[t][:, 128 * lt:128 * lt + 128],
                                            wvs[:, t, :],
                                            start=(t == 0), stop=(t == ET - 1))
                                    va3 = va_all[:, 520 * lt:520 * lt + 520] \
                                        .rearrange("p (h c) -> p h c", h=HC)
                                    nc.vector.tensor_copy(
                                        va3[:, 4 * vh:4 * vh + 4, 0:64],
                                        acc[:, 0:256].rearrange(
                                            "p (h c) -> p h c", h=4))
                                    if vh == 0:
                                        nc.vector.tensor_copy(
                                            va3[:, :, 64:65], ones_c)

                        # MLP up-projection + relu, staged to HBM scratch
                        with (
                            tc.tile_pool(name="pwi", bufs=3) as pwi,
                            tc.tile_pool(name="phT", bufs=6) as phT,
                        ):
                            wi3 = wi_d.rearrange("(et ep) f -> ep et f", ep=128)
                            for ft in range(FT):
                                wha = pwi.tile([128, ET // 2, 128], F32R,
                                               tag="wi", name="wha")
                                nc.sync.dma_start(
                                    wha[:],
                                    wi3[:, 0:ET // 2,
                                        128 * ft:128 * ft + 128])
                                whb = pwi.tile([128, ET // 2, 128], F32R,
                                               tag="wi", name="whb")
                                nc.sync.dma_start(
                                    whb[:],
                                    wi3[:, ET // 2:ET,
                                        128 * ft:128 * ft + 128])
                                for c in range(NSUP):
                                    acc = pps.tile([128, 512], F32, tag="ps")
                                    for t in range(ET):
                                        w_half = wha if t < ET // 2 else whb
                                        nc.tensor.matmul(
                                            acc[:],
                                            w_half[:, t % (ET // 2), :],
                                            xt[t][:, 512 * c:512 * c + 512],
                                            start=(t == 0), stop=(t == ET - 1))
                                    hst = phT.tile([128, 512], mdt, tag="hT")
                                    nc.scalar.activation(hst[:], acc[:], Act.Relu)
                                    nc.sync.dma_start(
                                        hT_d[ft, :, 512 * c:512 * c + 512], hst[:])

                    # ------------ phase 2: attention (emitted inside pqk) ----
                    with (
                        tc.tile_pool(name="poba", bufs=2) as poba,
                        tc.tile_pool(name="pband", bufs=1) as pband,
                        tc.tile_pool(name="pexp", bufs=6) as pexp,
                        tc.tile_pool(name="prr", bufs=2) as prr,
                        tc.tile_pool(name="prs", bufs=2) as prs,
                        tc.tile_pool(name="prb", bufs=2) as prb,
                        tc.tile_pool(name="pth", bufs=2) as pth,
                    ):
                        band_sb = pband.tile([128, HC * BAND_W], F32R, tag="band")
                        band3 = band_sb[:].rearrange("p (h w) -> p h w", h=HC)
                        ident = pband.tile([128, 128], F32R, tag="ident")
                        nc.sync.dma_start(ident[:], ident_d)
                        bfut_sb = pband.tile([128, HC], F32, tag="bfut")
                        nc.sync.dma_start(bfut_sb[:], bfut_d)
                        wo3 = wo_d.rearrange("(pt pp) e -> pp pt e", pp=128)
                        wos = pwo.tile([128, PT, E], F32R, tag="wo",
                                       name="wos")
                        prev = None
                        for s in range(NSUP):
                            qs = 512 * s
                            ktiles = 4 * (s + 1) if causal else LT
                            cts = [pct.tile([128, 512], F32R, tag="ct",
                                            name=f"ct{s}_{p}")
                                   for p in range(PT)]
                            def emit_ctx2(cps_, p_, kt, eas, off, stop):
                                for half in (0, 1):
                                    h = 2 * p_ + half
                                    base = 520 * kt + 65 * h
                                    nc.tensor.matmul(
                                        cps_[half][0:65, off:512],
                                        va_all[:, base:base + 65],
                                        eas[half][:, off:512],
                                        start=(kt == 0), stop=stop)

                            def normalize(p_, cps_):
                                # softmax denominator -> reciprocal ->
                                # broadcast -> scale, packing ct halves
                                ct = cts[p_]
                                for half, cp in ((0, cps_[0]), (1, cps_[1])):
                                    rr = prr.tile([128, 512], F32, tag="rr",
                                                  name="rr")
                                    nc.vector.reciprocal(rr[64:65, :],
                                                         cp[64:65, :])
                                    rs = prs.tile([1, 512], F32, tag="rs",
                                                  name="rs")
                                    nc.sync.dma_start(rs[0:1, :],
                                                      rr[64:65, :])
                                    rb = prb.tile([64, 512], F32, tag="rb",
                                                  name="rb")
                                    nc.gpsimd.partition_broadcast(
                                        rb[:], rs[0:1, :])
                                    if half == 0:
                                        nc.vector.tensor_tensor(
                                            ct[0:64, :], cp[0:64, :], rb[:],
                                            Alu.mult)
                                    else:
                                        th = pth.tile([64, 512], F32R,
                                                      tag="th", name="th")
                                        nc.vector.tensor_tensor(
                                            th[:], cp[0:64, :], rb[:],
                                            Alu.mult)
                                        nc.sync.dma_start(ct[64:128, :],
                                                          th[:])

                            def flush(carry):
                                cps_, p_, pd = carry
                                emit_ctx2(cps_, p_, pd[0], pd[1], pd[2],
                                          stop=True)
                                normalize(p_, cps_)
                                nonlocal prev
                                if prev is not None and p_ >= 1:
                                    sched = {1: [(0, 0), (0, 1), (1, 0)],
                                             2: [(1, 1), (2, 0), (2, 1)],
                                             3: [(3, 0), (3, 1)]}[p_]
                                    outproj(*prev, poba, sched)
                                    if p_ == 3:
                                        prev = None

                            carry = None
                            for p in range(PT):
                                if s == 0:
                                    for h in (2 * p, 2 * p + 1):
                                        nc.sync.dma_start(
                                            band_sb[:, BAND_W * h:
                                                    BAND_W * h + BAND_W],
                                            band_d[:, h, :])
                                cps = (pctx.tile([65, 512], F32, tag="ctx",
                                                 name=f"cpa{s}_{p}"),
                                       pctx.tile([65, 512], F32, tag="ctx",
                                                 name=f"cpb{s}_{p}"))
                                pend = None
                                for kt in range(ktiles):
                                    k0 = 128 * kt
                                    # causal: q-cols left of k0 are fully
                                    # masked for this k-tile -- skip them.
                                    # Cap at 256 so matmul free dims stay
                                    # >=256 (float32r runs 4x slower below);
                                    # the extra cols are band-masked anyway.
                                    off = (min(max(0, k0 - qs), 256)
                                           if causal else 0)
                                    o_lo = max(k0 - BAND_OFF, qs + off)
                                    o_hi = min(k0 + 256, qs + 512)
                                    has_band = o_hi > o_lo
                                    ul = (min(max(k0 - BAND_OFF - qs, 0), 512)
                                          if not causal else 0)
                                    full_fut = (not causal) and not has_band \
                                        and k0 > qs + 511
                                    eas = []
                                    for half, tp in ((0, (0, 0)), (1, (64, 0))):
                                        h = 2 * p + half
                                        ps_t = pps.tile([128, 512], F32,
                                                        tag="ps", name="sc")
                                        r0 = 64 * half
                                        nc.tensor.matmul(
                                            ps_t[:, off:512],
                                            kT[p][r0:r0 + 64, k0:k0 + 128],
                                            qT[p][r0:r0 + 64,
                                                  qs + off:qs + 512],
                                            start=True, stop=not has_band,
                                            tile_position=tp)
                                        if ul > 0 and not full_fut:
                                            nc.vector.tensor_scalar_add(
                                                ps_t[:, 0:ul], ps_t[:, 0:ul],
                                                bfut_sb[:, h:h + 1])
                                        if has_band:
                                            psl = slice(o_lo - qs, o_hi - qs)
                                            bsl = slice(
                                                o_lo - (k0 - BAND_OFF),
                                                o_hi - (k0 - BAND_OFF))
                                            nc.tensor.matmul(
                                                ps_t[:, psl], ident[:],
                                                band3[:, h, bsl],
                                                start=False, stop=True)
                                        ea = pexp.tile([128, 512], F32R,
                                                       tag="exp", name="ea")
                                        if full_fut:
                                            nc.scalar.activation(
                                                ea[:, off:512],
                                                ps_t[:, off:512], Act.Exp,
                                                bias=bfut_sb[:, h:h + 1])
                                        else:
                                            nc.scalar.activation(
                                                ea[:, off:512],
                                                ps_t[:, off:512], Act.Exp)
                                        eas.append(ea)
                                    if kt == 0 and carry is not None:
                                        flush(carry)
                                        carry = None
                                    if pend is not None:
                                        emit_ctx2(cps, p, pend[0], pend[1],
                                                  pend[2], stop=False)
                                    pend = (kt, eas, off)
                                carry = (cps, p, pend)
                            flush(carry)
                            if s == 0:
                                for p4 in range(PT):
                                    nc.sync.dma_start(wos[:, p4, :],
                                                      wo3[:, p4, :])
                            prev = (s, cts)
                        # final super's output projection, inside attention scope
                        outproj(*prev, poba,
                                [(qt, ec) for qt in range(4) for ec in range(2)],
                                use_act=True)

                # ---- phase 3: MLP down-projection (pqk/pva freed) -----------
                with (
                    tc.tile_pool(name="pwmo", bufs=FT) as pwmo,
                    tc.tile_pool(name="phin", bufs=3) as phin,
                    tc.tile_pool(name="pobb", bufs=2) as pobb,
                ):
                    hT3 = hT_d.rearrange("ft fp l -> fp ft l")
                    hins0 = phin.tile([128, FT, 128], mdt, tag="hin",
                                      name="hins0")
                    nc.sync.dma_start(hins0[:], hT3[:, :, 0:128])
                    wmo3 = wmo_d.rearrange("(ft fp) e -> fp ft e", fp=128)
                    wms = []
                    for ft in range(FT):
                        wm = pwmo.tile([128, E], mdt, tag="wmo",
                                       name=f"wm{ft}")
                        nc.sync.dma_start(wm[:], wmo3[:, ft, :])
                        wms.append(wm)

                    for lt in range(LT):
                        if lt == 0:
                            hins = hins0
                        else:
                            hins = phin.tile([128, FT, 128], mdt, tag="hin",
                                             name="hins")
                            nc.sync.dma_start(
                                hins[:], hT3[:, :, 128 * lt:128 * lt + 128])
                        for ec in range(2):
                            acc = pps.tile([128, 512], F32, tag="ps")
                            for ft in range(FT):
                                nc.tensor.matmul(
                                    acc[:], hins[:, ft, :],
                                    wms[ft][:, 512 * ec:512 * ec + 512],
                                    start=(ft == 0), stop=(ft == FT - 1))
                            ob = pobb.tile([128, 512], F32, tag="ob")
                            nc.scalar.copy(ob[:], acc[:])
                            nc.sync.dma_start(
                                mlp_d[128 * lt:128 * lt + 128,
                                      512 * ec:512 * ec + 512], ob[:])

    nc.compile()
    return nc


_NC_CACHE = {}


def _get_nc(causal: bool):
    if causal not in _NC_CACHE:
        _NC_CACHE[causal] = _build(causal)
    return _NC_CACHE[causal]


def _bucket(n):
    """T5 relative-position bucket (causal), exact numpy replica of the
    jax fp32 reference computation."""
    n = np.asarray(n)
    nf = np.maximum(n.astype(np.float32), np.float32(1.0))
    v = np.log(nf / np.float32(16.0)).astype(np.float32)
    v = (v / np.float32(np.log(8.0))) * np.float32(16.0)
    val_large = 16 + v.astype(np.int32)
    val_large = np.minimum(val_large, NUM_BUCKETS - 1)
    return np.where(n < 16, n, val_large)


def _make_band(rel_emb, heads, causal):
    """band[i, hl, j] = adjustment for distance d = j - BAND_OFF - i.

    d < 0   : -30000 (causal mask) or rel_emb[0]-rel_emb[31] (dense)
    0..112  : rel_emb[bucket(d)] - rel_emb[31]
    >= 113  : 0   (bucket 31 everywhere; constant per row cancels in softmax)
    """
    d = np.arange(-(BAND_OFF + 127), 256)          # all possible j - OFF - i
    pos = np.maximum(d, 0)
    bv = rel_emb[_bucket(pos)][:, heads] - rel_emb[NUM_BUCKETS - 1][heads]
    bv = np.where(d[:, None] >= 113, np.float32(0.0), bv)
    if causal:
        bv = np.where(d[:, None] < 0, np.float32(-30000.0), bv)
    else:
        fut = rel_emb[0][heads] - rel_emb[NUM_BUCKETS - 1][heads]
        bv = np.where(d[:, None] < 0, fut[None, :], bv)
    i = np.arange(128)[:, None]
    j = np.arange(BAND_W)[None, :]
    idx = (j - BAND_OFF - i) + (BAND_OFF + 127)
    return bv.astype(np.float32)[idx]          # [128, BAND_W, HC]


def _prep_in_maps(inputs, wq, wk, wv, wo, wi, wmo, rel_emb, decoder_mask):
    inputs = np.asarray(inputs, dtype=np.float32)
    wq = np.asarray(wq, dtype=np.float32)
    wk = np.asarray(wk, dtype=np.float32)
    wv = np.asarray(wv, dtype=np.float32)
    wo = np.asarray(wo, dtype=np.float32)
    wi = np.asarray(wi, dtype=np.float32)
    wmo = np.asarray(wmo, dtype=np.float32)
    rel_emb = np.asarray(rel_emb, dtype=np.float32)
    mask = np.asarray(decoder_mask).reshape(L, L)

    tril = np.tril(np.ones((L, L), dtype=bool))
    if np.array_equal(mask, tril):
        causal = True
    elif mask.all():
        causal = False
    else:
        raise NotImplementedError("only causal or all-true masks supported")

    in_maps = []
    for c in range(NCORES):
        b, g = divmod(c, 2)
        heads = np.arange(HC * g, HC * (g + 1))
        band = _make_band(rel_emb, heads, causal)       # [128, BAND_W, HC]
        band = np.ascontiguousarray(band.transpose(0, 2, 1))  # [128, HC, W]
        bfut = np.broadcast_to(
            (rel_emb[0][heads] - rel_emb[NUM_BUCKETS - 1][heads])
            .astype(np.float32), (128, HC)).copy()
        in_maps.append(dict(
            xT=np.ascontiguousarray(inputs[b].T),
            wq=np.ascontiguousarray(wq[:, heads, :]).reshape(E, HC * D),
            wk=np.ascontiguousarray(wk[:, heads, :]).reshape(E, HC * D),
            wv=np.ascontiguousarray(wv[:, heads, :]).reshape(E, HC * D),
            wo=np.ascontiguousarray(wo[heads]).reshape(HC * D, E),
            wi=np.ascontiguousarray(wi[:, FC * g:FC * (g + 1)]),
            wmo=(np.ascontiguousarray(wmo[FC * g:FC * (g + 1), :])
                 .astype(ml_dtypes.bfloat16) if USE_BF16_MLP2 else
                 np.ascontiguousarray(wmo[FC * g:FC * (g + 1), :])),
            band=band,
            bfut=bfut,
            ident=np.eye(128, dtype=np.float32),
        ))
    return in_maps, causal, inputs


def run(trace=False, **kw):
    in_maps, causal, inputs = _prep_in_maps(**kw)
    nc = _get_nc(causal)
    res = run_bass_kernel_spmd(nc, in_maps, list(range(NCORES)), trace=trace)
    out = np.empty((B, L, E), dtype=np.float32)
    for b in range(B):
        out[b] = (inputs[b]
                  + res.results[2 * b]["attn_out"]
                  + res.results[2 * b]["mlp_out"]
                  + res.results[2 * b + 1]["attn_out"]
                  + res.results[2 * b + 1]["mlp_out"])
    return out, res


def kernel(**inputs):
    out, _ = run(**inputs)
    return out



# revision 4
# speedup vs baseline: 1.3639x; 1.3639x over previous
"""Trainium2 Bass kernel for nn_DecoderLayer_19851338842283.

8 cores: data-parallel over batch (4) x tensor-parallel (2) over heads/mlp_dim.
fp8(e4m3) DoubleRow matmuls for projections / MLP (with host-side residual
weight passes for accuracy), fp8-DR d-split scores, bf16 exp/ctx/out-proj.
Host sums the two tensor-parallel partials and adds the residual.
"""

import ml_dtypes
import numpy as np

import concourse.bacc as bacc
import concourse.mybir as mybir
import concourse.tile as tile
from concourse.bass_utils import run_bass_kernel_spmd

F32 = mybir.dt.float32
BF16 = mybir.dt.bfloat16
FP8 = mybir.dt.float8e4
Act = mybir.ActivationFunctionType
Alu = mybir.AluOpType
DR = mybir.MatmulPerfMode.DoubleRow
E4 = ml_dtypes.float8_e4m3

B, L, E, H, D, F = 4, 2048, 1024, 16, 64, 4096
HC = H // 2          # heads per core = 8
FC = F // 2          # mlp dim per core = 2048
NCORES = 8
ET = E // 128        # 8
LT = L // 128        # 16
FT = FC // 128       # 16
NSUP = L // 512      # 4
TB = E // 256        # 4 DR pair-blocks over E
PB = FC // 256       # 8 DR pair-blocks over FC
BAND_OFF = 128
BAND_W = 384
NUM_BUCKETS = 32
QSC = np.float32(0.125 ** 0.5 / 16.0)   # psum->q8/k8 copy scale


def _build(causal: bool):
    nc = bacc.Bacc("TRN2", target_bir_lowering=False, debug=False,
                   num_devices=NCORES)
    x8_d = nc.dram_tensor("x8", [128, ET, L], FP8, kind="ExternalInput").ap()
    xr_d = nc.dram_tensor("xr", [128, ET, L], FP8, kind="ExternalInput").ap()
    wqs_d = nc.dram_tensor("wqs", [3, 4, 128, TB, 2, 128], FP8,
                           kind="ExternalInput").ap()
    wks_d = nc.dram_tensor("wks", [3, 4, 128, TB, 2, 128], FP8,
                           kind="ExternalInput").ap()
    wvm_d = nc.dram_tensor("wvm", [3, 2, 128, TB, 2, 256], FP8,
                           kind="ExternalInput").ap()
    wis_d = nc.dram_tensor("wis", [3, FT, 128, TB, 2, 128], FP8,
                           kind="ExternalInput").ap()
    wmm_d = nc.dram_tensor("wmm", [2, 128, FT, E], FP8,
                           kind="ExternalInput").ap()
    wos_d = nc.dram_tensor("wos", [128, 4, E], BF16, kind="ExternalInput").ap()
    band_d = nc.dram_tensor("band", [128, HC, BAND_W], BF16,
                            kind="ExternalInput").ap()
    ident_d = nc.dram_tensor("ident", [128, 128], BF16,
                             kind="ExternalInput").ap()
    bfut_d = nc.dram_tensor("bfut", [128, HC], F32, kind="ExternalInput").ap()
    attn_d = nc.dram_tensor("attn_out", [L, E], F32, kind="ExternalOutput").ap()
    mlp_d = nc.dram_tensor("mlp_out", [L, E], F32, kind="ExternalOutput").ap()

    with tile.TileContext(nc) as tc:
        with (
            tc.tile_pool(name="pbig", bufs=1) as pbig,
            tc.tile_pool(name="pqk", bufs=4) as pqk,
            tc.tile_pool(name="pva", bufs=1) as pva,
        ):
            x8 = pbig.tile([128, ET, L], FP8, tag="x8", name="x8")
            xr = pbig.tile([128, ET, L], FP8, tag="xr", name="xr")
            h8 = pbig.tile([128, FT, L], FP8, tag="h8", name="h8")
            q8s = [pqk.tile([128, 2, L], FP8, tag="qk", name=f"q8_{g}")
                   for g in range(2)]
            k8s = [pqk.tile([128, 2, L], FP8, tag="qk", name=f"k8_{g}")
                   for g in range(2)]
            va_all = pva.tile([128, LT * HC * 65], BF16, tag="va", name="va")

            for t in range(ET):
                nc.sync.dma_start(x8[:, t, :], x8_d[:, t, :])
                nc.sync.dma_start(xr[:, t, :], xr_d[:, t, :])

            def xmov(ps, tb, c0, cw):
                src = x8 if ps < 2 else xr
                return src[:, 2 * tb:2 * tb + 2, c0:c0 + cw]

            # ---------------- q/k projections -------------------------------
            with (
                tc.tile_pool(name="pw", bufs=6) as pw,
                tc.tile_pool(name="pps1", bufs=3, space="PSUM") as pps1,
            ):
                for w_d, dsts in ((wqs_d, q8s), (wks_d, k8s)):
                    for tl in range(4):      # (g, dpair)
                        g, dp = divmod(tl, 2)
                        sts = []
                        for ps in range(3):
                            st = pw.tile([128, TB, 2, 128], FP8, tag="w",
                                         name="st")
                            nc.sync.dma_start(st[:], w_d[ps, tl])
                            sts.append(st)
                        for c in range(NSUP):
                            acc = pps1.tile([128, 512], F32, tag="ps1")
                            for ps in range(3):
                                for tb in range(TB):
                                    nc.tensor.matmul(
                                        acc[:], sts[ps][:, tb],
                                        xmov(ps, tb, 512 * c, 512),
                                        start=(ps == 0 and tb == 0),
                                        stop=(ps == 2 and tb == TB - 1),
                                        perf_mode=DR)
                            nc.scalar.mul(
                                dsts[g][:, dp, 512 * c:512 * c + 512],
                                acc[:], float(QSC))

                # ---------------- v projection (bf16 va + ones) -------------
                ones_c = nc.const_aps.tensor(1.0, [128, HC, 1], BF16)
                for vh in range(2):
                    wvs = []
                    for ps in range(3):
                        wv = pw.tile([128, TB, 2, 256], FP8, tag="w",
                                     name="wv")
                        nc.sync.dma_start(wv[:], wvm_d[ps, vh])
                        wvs.append(wv)
                    for lt in range(LT):
                        acc = pps1.tile([128, 256], F32, tag="ps1")
                        for ps in range(3):
                            for tb in range(TB):
                                nc.tensor.matmul(
                                    acc[:], xmov(ps, tb, 128 * lt, 128),
                                    wvs[ps][:, tb],
                                    start=(ps == 0 and tb == 0),
                                    stop=(ps == 2 and tb == TB - 1),
                                    perf_mode=DR)
                        va3 = va_all[:, 520 * lt:520 * lt + 520] \
                            .rearrange("p (h c) -> p h c", h=HC)
                        nc.vector.tensor_scalar_mul(
                            va3[:, 4 * vh:4 * vh + 4, 0:64],
                            acc[:].rearrange("p (h c) -> p h c", h=4),
                            1.0 / 16.0)
                        if vh == 0:
                            nc.vector.tensor_copy(va3[:, :, 64:65], ones_c)

            # -------- attention (+ interleaved MLP-in) ----------------------
            with (
                tc.tile_pool(name="pwi", bufs=8) as pwi,
                tc.tile_pool(name="pband", bufs=1) as pband,
                tc.tile_pool(name="pct", bufs=8) as pct,
                tc.tile_pool(name="pwo", bufs=1) as pwo,
                tc.tile_pool(name="pexp", bufs=4) as pexp,
                tc.tile_pool(name="prr", bufs=2) as prr,
                tc.tile_pool(name="prs", bufs=2) as prs,
                tc.tile_pool(name="prb", bufs=2) as prb,
                tc.tile_pool(name="pth", bufs=2) as pth,
                tc.tile_pool(name="poba", bufs=2) as poba,
                tc.tile_pool(name="pps", bufs=3, space="PSUM") as pps,
                tc.tile_pool(name="pctx", bufs=2, space="PSUM") as pctx,
                tc.tile_pool(name="pout", bufs=1, space="PSUM") as pout,
                tc.tile_pool(name="pmps", bufs=2, space="PSUM") as pmps,
            ):
                band_sb = pband.tile([128, HC * BAND_W], BF16, tag="band")
                band3 = band_sb[:].rearrange("p (h w) -> p h w", h=HC)
                ident = pband.tile([128, 128], BF16, tag="ident")
                nc.sync.dma_start(ident[:], ident_d)
                bfut_sb = pband.tile([128, HC], F32, tag="bfut")
                nc.sync.dma_start(bfut_sb[:], bfut_d)
                for hh in range(HC):
                    nc.sync.dma_start(
                        band_sb[:, BAND_W * hh:BAND_W * hh + BAND_W],
                        band_d[:, hh, :])
                wos = pwo.tile([128, 4, E], BF16, tag="wo", name="wos")
                nc.sync.dma_start(wos[:], wos_d)

                mlp_units = [(ft, c) for ft in range(FT) for c in range(NSUP)]
                mlp_i = 0
                wi_sts = {}

                def emit_mlp_unit():
                    nonlocal mlp_i
                    if mlp_i >= len(mlp_units):
                        return
                    ft, c = mlp_units[mlp_i]
                    mlp_i += 1
                    if c == 0:
                        sts = []
                        for ps in range(3):
                            st = pwi.tile([128, TB, 2, 128], FP8, tag="wi",
                                          name="wist")
                            nc.sync.dma_start(st[:], wis_d[ps, ft])
                            sts.append(st)
                        wi_sts[ft] = sts
                    sts = wi_sts[ft]
                    acc = pmps.tile([128, 512], F32, tag="mps")
                    for ps in range(3):
                        for tb in range(TB):
                            nc.tensor.matmul(
                                acc[:], sts[ps][:, tb],
                                xmov(ps, tb, 512 * c, 512),
                                start=(ps == 0 and tb == 0),
                                stop=(ps == 2 and tb == TB - 1),
                                perf_mode=DR)
                    nc.vector.tensor_scalar(
                        out=h8[:, ft, 512 * c:512 * c + 512], in0=acc[:],
                        scalar1=1.0 / 16.0, scalar2=0.0,
                        op0=Alu.mult, op1=Alu.max)

                def outproj(s_prev, cts_prev, chunks, last=False):
                    for qt, ec in chunks:
                        acc = pout.tile([128, 512], F32, tag="out",
                                        name="opacc")
                        for p in range(4):
                            nc.tensor.matmul(
                                acc[:],
                                cts_prev[p][:, 128 * qt:128 * qt + 128],
                                wos[:, p, 512 * ec:512 * ec + 512],
                                start=(p == 0), stop=(p == 3))
                        ob = poba.tile([128, 512], F32, tag="ob")
                        if last:
                            nc.scalar.copy(ob[:], acc[:])
                        else:
                            nc.vector.tensor_copy(ob[:], acc[:])
                        qs0 = 512 * s_prev
                        nc.sync.dma_start(
                            attn_d[qs0 + 128 * qt:qs0 + 128 * qt + 128,
                                   512 * ec:512 * ec + 512], ob[:])

                prev = None
                for s in range(NSUP):
                    qs = 512 * s
                    ktiles = 4 * (s + 1) if causal else LT
                    cts = [pct.tile([128, 512], BF16, tag="ct",
                                    name=f"ct{s}_{p}") for p in range(4)]
                    for h in range(HC):
                        g, u = divmod(h, 4)
                        p = h // 2
                        cps = pctx.tile([65, 512], F32, tag="ctx",
                                        name=f"cps{s}_{h}")
                        pend = None
                        for kt in range(ktiles):
                            k0 = 128 * kt
                            off = min(max(0, k0 - qs), 384) if causal else 0
                            o_lo = max(k0 - BAND_OFF, qs + off)
                            o_hi = min(k0 + 256, qs + 512)
                            has_band = o_hi > o_lo
                            ul = (min(max(k0 - BAND_OFF - qs, 0), 512)
                                  if not causal else 0)
                            ps_t = pps.tile([128, 512], F32, tag="ps",
                                            name="sc")
                            nc.tensor.matmul(
                                ps_t[:, off:512],
                                k8s[g][32 * u:32 * u + 32, :, k0:k0 + 128],
                                q8s[g][32 * u:32 * u + 32, :,
                                       qs + off:qs + 512],
                                start=True, stop=not has_band,
                                perf_mode=DR, tile_position=(32 * u, 0))
                            if has_band:
                                psl = slice(o_lo - qs, o_hi - qs)
                                bsl = slice(o_lo - (k0 - BAND_OFF),
                                            o_hi - (k0 - BAND_OFF))
                                nc.tensor.matmul(
                                    ps_t[:, psl], ident[:],
                                    band3[:, h, bsl],
                                    start=False, stop=True)
                            if ul > 0:
                                nc.vector.tensor_scalar_add(
                                    ps_t[:, 0:ul], ps_t[:, 0:ul],
                                    bfut_sb[:, h:h + 1])
                            ea = pexp.tile([128, 512], BF16, tag="exp",
                                           name="ea")
                            nc.scalar.activation(ea[:, off:512],
                                                 ps_t[:, off:512], Act.Exp)
                            if pend is not None:
                                k0p, eap, offp = pend
                                nc.tensor.matmul(
                                    cps[0:65, offp:512],
                                    va_all[:, 520 * (k0p // 128) + 65 * h:
                                           520 * (k0p // 128) + 65 * h + 65],
                                    eap[:, offp:512],
                                    start=(k0p == 0), stop=False)
                            pend = (k0, ea, off)
                        k0p, eap, offp = pend
                        nc.tensor.matmul(
                            cps[0:65, offp:512],
                            va_all[:, 520 * (k0p // 128) + 65 * h:
                                   520 * (k0p // 128) + 65 * h + 65],
                            eap[:, offp:512],
                            start=(k0p == 0), stop=True)
                        # normalize head h -> ct[p] bf16
                        rr = prr.tile([65, 512], F32, tag="rr", name="rr")
                        nc.vector.reciprocal(rr[64:65, :], cps[64:65, :])
                        rs = prs.tile([1, 512], F32, tag="rs", name="rs")
                        nc.sync.dma_start(rs[0:1, :], rr[64:65, :])
                        rb = prb.tile([64, 512], F32, tag="rb", name="rb")
                        nc.gpsimd.partition_broadcast(rb[:], rs[0:1, :])
                        if h % 2 == 0:
                            nc.vector.tensor_tensor(
                                cts[p][0:64, :], cps[0:64, :], rb[:],
                                Alu.mult)
                        else:
                            th = pth.tile([64, 512], BF16, tag="th",
                                          name="th")
                            nc.vector.tensor_tensor(
                                th[:], cps[0:64, :], rb[:], Alu.mult)
                            nc.sync.dma_start(cts[p][64:128, :], th[:])
                        emit_mlp_unit()
                        emit_mlp_unit()
                        if prev is not None:
                            outproj(prev[0], prev[1], [divmod(h, 2)])
                            if h == HC - 1:
                                prev = None
                    prev = (s, cts)
                outproj(prev[0], prev[1],
                        [(qt, ec) for qt in range(4) for ec in range(2)],
                        last=True)
                while mlp_i < len(mlp_units):
                    emit_mlp_unit()

            # ---------------- MLP down-projection ---------------------------
            with (
                tc.tile_pool(name="pwm", bufs=2) as pwm,
                tc.tile_pool(name="pobb", bufs=2) as pobb,
                tc.tile_pool(name="pps3", bufs=3, space="PSUM") as pps3,
            ):
                wms = []
                for ps in range(2):
                    wm = pwm.tile([128, FT, E], FP8, tag="wm", name=f"wm{ps}")
                    for ft in range(FT):
                        nc.sync.dma_start(wm[:, ft, :], wmm_d[ps, :, ft, :])
                    wms.append(wm)
                h4 = h8[:].rearrange("p (j pb) l -> p j pb l", j=2)
                wm4 = [w[:].rearrange("p (j pb) e -> p j pb e", j=2)
                       for w in wms]
                for lt in range(LT):
                    for ec in range(2):
                        acc = pps3.tile([128, 512], F32, tag="ps3")
                        for ps in range(2):
                            for pb in range(PB):
                                nc.tensor.matmul(
                                    acc[:],
                                    h4[:, :, pb, 128 * lt:128 * lt + 128],
                                    wm4[ps][:, :, pb,
                                            512 * ec:512 * ec + 512],
                                    start=(ps == 0 and pb == 0),
                                    stop=(ps == 1 and pb == PB - 1),
                                    perf_mode=DR)
                        ob = pobb.tile([128, 512], F32, tag="ob")
                        nc.scalar.mul(ob[:], acc[:], 1.0 / 32.0)
                        nc.sync.dma_start(
                            mlp_d[128 * lt:128 * lt + 128,
                                  512 * ec:512 * ec + 512], ob[:])

    nc.compile()
    return nc


_NC_CACHE = {}


def _get_nc(causal: bool):
    if causal not in _NC_CACHE:
        _NC_CACHE[causal] = _build(causal)
    return _NC_CACHE[causal]


def _bucket(n):
    n = np.asarray(n)
    nf = np.maximum(n.astype(np.float32), np.float32(1.0))
    v = np.log(nf / np.float32(16.0)).astype(np.float32)
    v = (v / np.float32(np.log(8.0))) * np.float32(16.0)
    val_large = np.minimum(16 + v.astype(np.int32), NUM_BUCKETS - 1)
    return np.where(n < 16, n, val_large)


def _make_band(rel_emb, heads, causal):
    d = np.arange(-(BAND_OFF + 127), 256)
    pos = np.maximum(d, 0)
    bv = rel_emb[_bucket(pos)][:, heads] - rel_emb[NUM_BUCKETS - 1][heads]
    bv = np.where(d[:, None] >= 113, np.float32(0.0), bv)
    if causal:
        bv = np.where(d[:, None] < 0, np.float32(-30000.0), bv)
    else:
        fut = rel_emb[0][heads] - rel_emb[NUM_BUCKETS - 1][heads]
        bv = np.where(d[:, None] < 0, fut[None, :], bv)
    i = np.arange(128)[:, None]
    j = np.arange(BAND_W)[None, :]
    idx = (j - BAND_OFF - i) + (BAND_OFF + 127)
    return bv.astype(np.float32)[idx]          # [128, BAND_W, HC]


def _f8(a):
    return np.ascontiguousarray(a, dtype=np.float32).astype(E4)


def _split16(w, s):
    """-> (e4m3(s*w), e4m3(s*w - f32(e4m3(s*w))), e4m3(f32(e4m3(s*w))/s))"""
    w = np.asarray(w, np.float32)
    s1 = _f8(s * w)
    f1 = s1.astype(np.float32)
    s2 = _f8(s * w - f1)
    s3 = _f8(f1 / s)
    return s1, s2, s3


def _stat_qk(w_c):
    """w_c [E, HC, D] -> [4(tile), TB, 128, 2, 128] in f32 (pre-quant)."""
    arr = w_c.reshape(E, 2, 4, 2, 32)           # e, g, u, dp, dm
    out = np.empty((4, TB, 128, 2, 128), np.float32)  # transposed on return
    for tl in range(4):
        g, dp = divmod(tl, 2)
        M = arr[:, g, :, dp, :].reshape(E, 128)  # m = 32u + dm
        out[tl] = M.reshape(TB, 2, 128, 128).transpose(0, 2, 1, 3)
    return out.transpose(0, 2, 1, 3, 4)


def _prep_in_maps(inputs, wq, wk, wv, wo, wi, wmo, rel_emb, decoder_mask):
    inputs = np.asarray(inputs, dtype=np.float32)
    wq = np.asarray(wq, dtype=np.float32)
    wk = np.asarray(wk, dtype=np.float32)
    wv = np.asarray(wv, dtype=np.float32)
    wo = np.asarray(wo, dtype=np.float32)
    wi = np.asarray(wi, dtype=np.float32)
    wmo = np.asarray(wmo, dtype=np.float32)
    rel_emb = np.asarray(rel_emb, dtype=np.float32)
    mask = np.asarray(decoder_mask).reshape(L, L)

    tril = np.tril(np.ones((L, L), dtype=bool))
    if np.array_equal(mask, tril):
        causal = True
    elif mask.all():
        causal = False
    else:
        raise NotImplementedError("only causal or all-true masks supported")

    in_maps = []
    for c in range(NCORES):
        b, g = divmod(c, 2)
        heads = np.arange(HC * g, HC * (g + 1))
        band = _make_band(rel_emb, heads, causal)        # [128, W, HC]
        band = np.ascontiguousarray(band.transpose(0, 2, 1)).astype(
            ml_dtypes.bfloat16)
        bfut = np.broadcast_to(
            (rel_emb[0][heads] - rel_emb[NUM_BUCKETS - 1][heads])
            .astype(np.float32), (128, HC)).copy()

        xT = inputs[b].T                                  # [E, L]
        x8 = _f8(xT)
        xr = _f8(16.0 * (xT - x8.astype(np.float32)))
        x8 = x8.reshape(ET, 128, L).transpose(1, 0, 2)    # [128, ET, L]
        xr = xr.reshape(ET, 128, L).transpose(1, 0, 2)

        wq_c = wq[:, heads, :]
        wk_c = wk[:, heads, :]
        wqs = np.stack(_split16(_stat_qk(wq_c), 16.0))    # [3,4,TB,128,2,128]
        wks = np.stack(_split16(_stat_qk(wk_c), 16.0))

        wv_c = wv[:, heads, :].reshape(E, HC * D)
        wvm = np.empty((3, 2, 128, TB, 2, 256), E4)
        for vh in range(2):
            N = wv_c[:, 256 * vh:256 * vh + 256]
            N = N.reshape(TB, 2, 128, 256).transpose(2, 0, 1, 3)
            s1, s2, s3 = _split16(N, 16.0)
            wvm[0, vh], wvm[1, vh], wvm[2, vh] = s1, s2, s3

        wi_c = wi[:, FC * g:FC * (g + 1)]
        wis = np.empty((3, FT, 128, TB, 2, 128), E4)
        for ft in range(FT):
            M = wi_c[:, 128 * ft:128 * ft + 128]
            M = M.reshape(TB, 2, 128, 128).transpose(2, 0, 1, 3)
            s1, s2, s3 = _split16(M, 16.0)
            wis[0, ft], wis[1, ft], wis[2, ft] = s1, s2, s3

        wmo_c = wmo[FC * g:FC * (g + 1), :]               # [FC, E]
        wm = wmo_c.reshape(FT, 128, E).transpose(1, 0, 2)  # [128, FT, E]
        m1 = _f8(32.0 * wm)
        m2 = _f8(32.0 * wm - m1.astype(np.float32))
        wmm = np.stack([m1, m2])

        wo_c = wo[heads]                                   # [HC, D, E]
        wos = wo_c.reshape(4, 2, 64, E).transpose(0, 1, 2, 3) \
            .reshape(4, 128, E).transpose(1, 0, 2)         # [128, 4, E]
        wos = np.ascontiguousarray(wos).astype(ml_dtypes.bfloat16)

        in_maps.append(dict(
            x8=np.ascontiguousarray(x8), xr=np.ascontiguousarray(xr),
            wqs=wqs, wks=wks, wvm=wvm, wis=wis, wmm=wmm,
            wos=wos, band=band,
            ident=np.eye(128, dtype=np.float32).astype(ml_dtypes.bfloat16),
            bfut=bfut,
        ))
    return in_maps, causal, inputs


def run(trace=False, **kw):
    in_maps, causal, inputs = _prep_in_maps(**kw)
    nc = _get_nc(causal)
    res = run_bass_kernel_spmd(nc, in_maps, list(range(NCORES)), trace=trace)
    out = np.empty((B, L, E), dtype=np.float32)
    for b in range(B):
        out[b] = (inputs[b]
                  + res.results[2 * b]["attn_out"]
                  + res.results[2 * b]["mlp_out"]
                  + res.results[2 * b + 1]["attn_out"]
                  + res.results[2 * b + 1]["mlp_out"])
    return out, res


def kernel(**inputs):
    out, _ = run(**inputs)
    return out


# revision 5
# speedup vs baseline: 1.3757x; 1.0086x over previous
"""Trainium2 Bass kernel for nn_DecoderLayer_19851338842283.

8 cores: data-parallel over batch (4) x tensor-parallel (2) over heads/mlp_dim.
fp8(e4m3) DoubleRow matmuls for projections / MLP (with host-side residual
weight passes for accuracy), fp8-DR d-split scores, bf16 exp/ctx/out-proj.
Host sums the two tensor-parallel partials and adds the residual.
"""

import ml_dtypes
import numpy as np

import concourse.bacc as bacc
import concourse.mybir as mybir
import concourse.tile as tile
from concourse.bass_utils import run_bass_kernel_spmd

F32 = mybir.dt.float32
BF16 = mybir.dt.bfloat16
FP8 = mybir.dt.float8e4
Act = mybir.ActivationFunctionType
Alu = mybir.AluOpType
DR = mybir.MatmulPerfMode.DoubleRow
E4 = ml_dtypes.float8_e4m3

B, L, E, H, D, F = 4, 2048, 1024, 16, 64, 4096
HC = H // 2          # heads per core = 8
FC = F // 2          # mlp dim per core = 2048
NCORES = 8
ET = E // 128        # 8
LT = L // 128        # 16
FT = FC // 128       # 16
NSUP = L // 512      # 4
TB = E // 256        # 4 DR pair-blocks over E
PB = FC // 256       # 8 DR pair-blocks over FC
BAND_OFF = 128
BAND_W = 384
NUM_BUCKETS = 32
QSC = np.float32(0.125 ** 0.5 / 16.0)   # psum->q8/k8 copy scale


def _build(causal: bool):
    nc = bacc.Bacc("TRN2", target_bir_lowering=False, debug=False,
                   num_devices=NCORES)
    x8_d = nc.dram_tensor("x8", [128, ET, L], FP8, kind="ExternalInput").ap()
    xr_d = nc.dram_tensor("xr", [128, ET, L], FP8, kind="ExternalInput").ap()
    wqs_d = nc.dram_tensor("wqs", [3, 4, 128, TB, 2, 128], FP8,
                           kind="ExternalInput").ap()
    wks_d = nc.dram_tensor("wks", [3, 4, 128, TB, 2, 128], FP8,
                           kind="ExternalInput").ap()
    wvm_d = nc.dram_tensor("wvm", [3, 2, 128, TB, 2, 256], FP8,
                           kind="ExternalInput").ap()
    wis_d = nc.dram_tensor("wis", [3, FT, 128, TB, 2, 128], FP8,
                           kind="ExternalInput").ap()
    wmm_d = nc.dram_tensor("wmm", [2, 128, FT, E], FP8,
                           kind="ExternalInput").ap()
    wos_d = nc.dram_tensor("wos", [128, 4, E], BF16, kind="ExternalInput").ap()
    band_d = nc.dram_tensor("band", [128, HC, BAND_W], BF16,
                            kind="ExternalInput").ap()
    ident_d = nc.dram_tensor("ident", [128, 128], BF16,
                             kind="ExternalInput").ap()
    bfut_d = nc.dram_tensor("bfut", [128, HC], F32, kind="ExternalInput").ap()
    attn_d = nc.dram_tensor("attn_out", [L, E], F32, kind="ExternalOutput").ap()
    mlp_d = nc.dram_tensor("mlp_out", [L, E], F32, kind="ExternalOutput").ap()

    with tile.TileContext(nc) as tc:
        with (
            tc.tile_pool(name="pbig", bufs=1) as pbig,
            tc.tile_pool(name="pqk", bufs=4) as pqk,
            tc.tile_pool(name="pva", bufs=1) as pva,
        ):
            x8 = pbig.tile([128, ET, L], FP8, tag="x8", name="x8")
            xr = pbig.tile([128, ET, L], FP8, tag="xr", name="xr")
            h8 = pbig.tile([128, FT, L], FP8, tag="h8", name="h8")
            q8s = [pqk.tile([128, 2, L], FP8, tag="qk", name=f"q8_{g}")
                   for g in range(2)]
            k8s = [pqk.tile([128, 2, L], FP8, tag="qk", name=f"k8_{g}")
                   for g in range(2)]
            va_all = pva.tile([128, LT * HC * 65], BF16, tag="va", name="va")

            for t in range(ET):
                nc.sync.dma_start(x8[:, t, :], x8_d[:, t, :])
                nc.sync.dma_start(xr[:, t, :], xr_d[:, t, :])

            def xmov(ps, tb, c0, cw):
                src = x8 if ps < 2 else xr
                return src[:, 2 * tb:2 * tb + 2, c0:c0 + cw]

            # ---------------- q/k projections -------------------------------
            with (
                tc.tile_pool(name="pw", bufs=6) as pw,
                tc.tile_pool(name="pps1", bufs=3, space="PSUM") as pps1,
            ):
                for w_d, dsts in ((wqs_d, q8s), (wks_d, k8s)):
                    for tl in range(4):      # (g, dpair)
                        g, dp = divmod(tl, 2)
                        sts = []
                        for ps in range(3):
                            st = pw.tile([128, TB, 2, 128], FP8, tag="w",
                                         name="st")
                            nc.sync.dma_start(st[:], w_d[ps, tl])
                            sts.append(st)
                        for c in range(NSUP):
                            acc = pps1.tile([128, 512], F32, tag="ps1")
                            for ps in range(3):
                                for tb in range(TB):
                                    nc.tensor.matmul(
                                        acc[:], sts[ps][:, tb],
                                        xmov(ps, tb, 512 * c, 512),
                                        start=(ps == 0 and tb == 0),
                                        stop=(ps == 2 and tb == TB - 1),
                                        perf_mode=DR)
                            nc.scalar.mul(
                                dsts[g][:, dp, 512 * c:512 * c + 512],
                                acc[:], float(QSC))

                # ---------------- v projection (bf16 va + ones) -------------
                ones_c = nc.const_aps.tensor(1.0, [128, HC, 1], BF16)
                for vh in range(2):
                    wvs = []
                    for ps in range(3):
                        wv = pw.tile([128, TB, 2, 256], FP8, tag="w",
                                     name="wv")
                        nc.sync.dma_start(wv[:], wvm_d[ps, vh])
                        wvs.append(wv)
                    for lt in range(LT):
                        acc = pps1.tile([128, 256], F32, tag="ps1")
                        for ps in range(3):
                            for tb in range(TB):
                                nc.tensor.matmul(
                                    acc[:], xmov(ps, tb, 128 * lt, 128),
                                    wvs[ps][:, tb],
                                    start=(ps == 0 and tb == 0),
                                    stop=(ps == 2 and tb == TB - 1),
                                    perf_mode=DR)
                        va3 = va_all[:, 520 * lt:520 * lt + 520] \
                            .rearrange("p (h c) -> p h c", h=HC)
                        nc.vector.tensor_scalar_mul(
                            va3[:, 4 * vh:4 * vh + 4, 0:64],
                            acc[:].rearrange("p (h c) -> p h c", h=4),
                            1.0 / 16.0)
                        if vh == 0:
                            nc.vector.tensor_copy(va3[:, :, 64:65], ones_c)

            # -------- attention (+ interleaved MLP-in) ----------------------
            with (
                tc.tile_pool(name="pwi", bufs=8) as pwi,
                tc.tile_pool(name="pband", bufs=1) as pband,
                tc.tile_pool(name="pct", bufs=8) as pct,
                tc.tile_pool(name="pwo", bufs=1) as pwo,
                tc.tile_pool(name="pexp", bufs=4) as pexp,
                tc.tile_pool(name="prr", bufs=2) as prr,
                tc.tile_pool(name="prs", bufs=2) as prs,
                tc.tile_pool(name="prb", bufs=2) as prb,
                tc.tile_pool(name="pth", bufs=2) as pth,
                tc.tile_pool(name="poba", bufs=2) as poba,
                tc.tile_pool(name="pps", bufs=3, space="PSUM") as pps,
                tc.tile_pool(name="pctx", bufs=2, space="PSUM") as pctx,
                tc.tile_pool(name="pout", bufs=1, space="PSUM") as pout,
                tc.tile_pool(name="pmps", bufs=2, space="PSUM") as pmps,
            ):
                band_sb = pband.tile([128, HC * BAND_W], BF16, tag="band")
                band3 = band_sb[:].rearrange("p (h w) -> p h w", h=HC)
                ident = pband.tile([128, 128], BF16, tag="ident")
                nc.sync.dma_start(ident[:], ident_d)
                bfut_sb = pband.tile([128, HC], F32, tag="bfut")
                nc.sync.dma_start(bfut_sb[:], bfut_d)
                for hh in range(HC):
                    nc.sync.dma_start(
                        band_sb[:, BAND_W * hh:BAND_W * hh + BAND_W],
                        band_d[:, hh, :])
                wos = pwo.tile([128, 4, E], BF16, tag="wo", name="wos")
                nc.sync.dma_start(wos[:], wos_d)

                mlp_units = [(ft, c) for ft in range(FT) for c in range(NSUP)]
                mlp_i = 0
                wi_sts = {}

                def emit_mlp_unit():
                    nonlocal mlp_i
                    if mlp_i >= len(mlp_units):
                        return
                    ft, c = mlp_units[mlp_i]
                    mlp_i += 1
                    if c == 0:
                        sts = []
                        for ps in range(3):
                            st = pwi.tile([128, TB, 2, 128], FP8, tag="wi",
                                          name="wist")
                            nc.sync.dma_start(st[:], wis_d[ps, ft])
                            sts.append(st)
                        wi_sts[ft] = sts
                    sts = wi_sts[ft]
                    acc = pmps.tile([128, 512], F32, tag="mps")
                    for ps in range(3):
                        for tb in range(TB):
                            nc.tensor.matmul(
                                acc[:], sts[ps][:, tb],
                                xmov(ps, tb, 512 * c, 512),
                                start=(ps == 0 and tb == 0),
                                stop=(ps == 2 and tb == TB - 1),
                                perf_mode=DR)
                    nc.vector.tensor_scalar(
                        out=h8[:, ft, 512 * c:512 * c + 512], in0=acc[:],
                        scalar1=1.0 / 16.0, scalar2=0.0,
                        op0=Alu.mult, op1=Alu.max)

                def outproj(s_prev, cts_prev, chunks, last=False):
                    for qt, ec in chunks:
                        acc = pout.tile([128, 512], F32, tag="out",
                                        name="opacc")
                        for p in range(4):
                            nc.tensor.matmul(
                                acc[:],
                                cts_prev[p][:, 128 * qt:128 * qt + 128],
                                wos[:, p, 512 * ec:512 * ec + 512],
                                start=(p == 0), stop=(p == 3))
                        ob = poba.tile([128, 512], F32, tag="ob")
                        if last:
                            nc.scalar.copy(ob[:], acc[:])
                        else:
                            nc.vector.tensor_copy(ob[:], acc[:])
                        qs0 = 512 * s_prev
                        nc.sync.dma_start(
                            attn_d[qs0 + 128 * qt:qs0 + 128 * qt + 128,
                                   512 * ec:512 * ec + 512], ob[:])

                prev = None
                ktc = [0]
                for s in range(NSUP):
                    qs = 512 * s
                    ktiles = 4 * (s + 1) if causal else LT
                    cts = [pct.tile([128, 512], BF16, tag="ct",
                                    name=f"ct{s}_{p}") for p in range(4)]
                    for h in range(HC):
                        g, u = divmod(h, 4)
                        p = h // 2
                        cps = pctx.tile([65, 512], F32, tag="ctx",
                                        name=f"cps{s}_{h}")
                        pend = None
                        for kt in range(ktiles):
                            k0 = 128 * kt
                            off = min(max(0, k0 - qs), 384) if causal else 0
                            o_lo = max(k0 - BAND_OFF, qs + off)
                            o_hi = min(k0 + 256, qs + 512)
                            has_band = o_hi > o_lo
                            ul = (min(max(k0 - BAND_OFF - qs, 0), 512)
                                  if not causal else 0)
                            ps_t = pps.tile([128, 512], F32, tag="ps",
                                            name="sc")
                            nc.tensor.matmul(
                                ps_t[:, off:512],
                                k8s[g][32 * u:32 * u + 32, :, k0:k0 + 128],
                                q8s[g][32 * u:32 * u + 32, :,
                                       qs + off:qs + 512],
                                start=True, stop=not has_band,
                                perf_mode=DR, tile_position=(32 * u, 0))
                            if has_band:
                                psl = slice(o_lo - qs, o_hi - qs)
                                bsl = slice(o_lo - (k0 - BAND_OFF),
                                            o_hi - (k0 - BAND_OFF))
                                nc.tensor.matmul(
                                    ps_t[:, psl], ident[:],
                                    band3[:, h, bsl],
                                    start=False, stop=True)
                            if ul > 0:
                                nc.vector.tensor_scalar_add(
                                    ps_t[:, 0:ul], ps_t[:, 0:ul],
                                    bfut_sb[:, h:h + 1])
                            ea = pexp.tile([128, 512], BF16, tag="exp",
                                           name="ea")
                            nc.scalar.activation(ea[:, off:512],
                                                 ps_t[:, off:512], Act.Exp)
                            if pend is not None:
                                k0p, eap, offp = pend
                                nc.tensor.matmul(
                                    cps[0:65, offp:512],
                                    va_all[:, 520 * (k0p // 128) + 65 * h:
                                           520 * (k0p // 128) + 65 * h + 65],
                                    eap[:, offp:512],
                                    start=(k0p == 0), stop=False)
                            pend = (k0, ea, off)
                            ktc[0] += 1
                            if ktc[0] % 5 == 0:
                                emit_mlp_unit()
                        k0p, eap, offp = pend
                        nc.tensor.matmul(
                            cps[0:65, offp:512],
                            va_all[:, 520 * (k0p // 128) + 65 * h:
                                   520 * (k0p // 128) + 65 * h + 65],
                            eap[:, offp:512],
                            start=(k0p == 0), stop=True)
                        # normalize head h -> ct[p] bf16
                        rr = prr.tile([65, 512], F32, tag="rr", name="rr")
                        nc.vector.reciprocal(rr[64:65, :], cps[64:65, :])
                        rs = prs.tile([1, 512], F32, tag="rs", name="rs")
                        nc.sync.dma_start(rs[0:1, :], rr[64:65, :])
                        rb = prb.tile([64, 512], F32, tag="rb", name="rb")
                        nc.gpsimd.partition_broadcast(rb[:], rs[0:1, :])
                        if h % 2 == 0:
                            nc.vector.tensor_tensor(
                                cts[p][0:64, :], cps[0:64, :], rb[:],
                                Alu.mult)
                        else:
                            th = pth.tile([64, 512], BF16, tag="th",
                                          name="th")
                            nc.vector.tensor_tensor(
                                th[:], cps[0:64, :], rb[:], Alu.mult)
                            nc.sync.dma_start(cts[p][64:128, :], th[:])
                        if prev is not None:
                            outproj(prev[0], prev[1], [divmod(h, 2)])
                            if h == HC - 1:
                                prev = None
                    prev = (s, cts)
                outproj(prev[0], prev[1],
                        [(qt, ec) for qt in range(4) for ec in range(2)],
                        last=True)
                while mlp_i < len(mlp_units):
                    emit_mlp_unit()

            # ---------------- MLP down-projection ---------------------------
            with (
                tc.tile_pool(name="pwm", bufs=2) as pwm,
                tc.tile_pool(name="pobb", bufs=2) as pobb,
                tc.tile_pool(name="pps3", bufs=3, space="PSUM") as pps3,
            ):
                wms = []
                for ps in range(2):
                    wm = pwm.tile([128, FT, E], FP8, tag="wm", name=f"wm{ps}")
                    for ft in range(FT):
                        nc.sync.dma_start(wm[:, ft, :], wmm_d[ps, :, ft, :])
                    wms.append(wm)
                h4 = h8[:].rearrange("p (j pb) l -> p j pb l", j=2)
                wm4 = [w[:].rearrange("p (j pb) e -> p j pb e", j=2)
                       for w in wms]
                for lt in range(LT):
                    for ec in range(2):
                        acc = pps3.tile([128, 512], F32, tag="ps3")
                        for ps in range(2):
                            for pb in range(PB):
                                nc.tensor.matmul(
                                    acc[:],
                                    h4[:, :, pb, 128 * lt:128 * lt + 128],
                                    wm4[ps][:, :, pb,
                                            512 * ec:512 * ec + 512],
                                    start=(ps == 0 and pb == 0),
                                    stop=(ps == 1 and pb == PB - 1),
                                    perf_mode=DR)
                        ob = pobb.tile([128, 512], F32, tag="ob")
                        nc.scalar.mul(ob[:], acc[:], 1.0 / 32.0)
                        nc.sync.dma_start(
                            mlp_d[128 * lt:128 * lt + 128,
                                  512 * ec:512 * ec + 512], ob[:])

    nc.compile()
    return nc


_NC_CACHE = {}


def _get_nc(causal: bool):
    if causal not in _NC_CACHE:
        _NC_CACHE[causal] = _build(causal)
    return _NC_CACHE[causal]


def _bucket(n):
    n = np.asarray(n)
    nf = np.maximum(n.astype(np.float32), np.float32(1.0))
    v = np.log(nf / np.float32(16.0)).astype(np.float32)
    v = (v / np.float32(np.log(8.0))) * np.float32(16.0)
    val_large = np.minimum(16 + v.astype(np.int32), NUM_BUCKETS - 1)
    return np.where(n < 16, n, val_large)


def _make_band(rel_emb, heads, causal):
    d = np.arange(-(BAND_OFF + 127), 256)
    pos = np.maximum(d, 0)
    bv = rel_emb[_bucket(pos)][:, heads] - rel_emb[NUM_BUCKETS - 1][heads]
    bv = np.where(d[:, None] >= 113, np.float32(0.0), bv)
    if causal:
        bv = np.where(d[:, None] < 0, np.float32(-30000.0), bv)
    else:
        fut = rel_emb[0][heads] - rel_emb[NUM_BUCKETS - 1][heads]
        bv = np.where(d[:, None] < 0, fut[None, :], bv)
    i = np.arange(128)[:, None]
    j = np.arange(BAND_W)[None, :]
    idx = (j - BAND_OFF - i) + (BAND_OFF + 127)
    return bv.astype(np.float32)[idx]          # [128, BAND_W, HC]


def _f8(a):
    return np.ascontiguousarray(a, dtype=np.float32).astype(E4)


def _split16(w, s):
    """-> (e4m3(s*w), e4m3(s*w - f32(e4m3(s*w))), e4m3(f32(e4m3(s*w))/s))"""
    w = np.asarray(w, np.float32)
    s1 = _f8(s * w)
    f1 = s1.astype(np.float32)
    s2 = _f8(s * w - f1)
    s3 = _f8(f1 / s)
    return s1, s2, s3


def _stat_qk(w_c):
    """w_c [E, HC, D] -> [4(tile), TB, 128, 2, 128] in f32 (pre-quant)."""
    arr = w_c.reshape(E, 2, 4, 2, 32)           # e, g, u, dp, dm
    out = np.empty((4, TB, 128, 2, 128), np.float32)  # transposed on return
    for tl in range(4):
        g, dp = divmod(tl, 2)
        M = arr[:, g, :, dp, :].reshape(E, 128)  # m = 32u + dm
        out[tl] = M.reshape(TB, 2, 128, 128).transpose(0, 2, 1, 3)
    return out.transpose(0, 2, 1, 3, 4)


def _prep_in_maps(inputs, wq, wk, wv, wo, wi, wmo, rel_emb, decoder_mask):
    inputs = np.asarray(inputs, dtype=np.float32)
    wq = np.asarray(wq, dtype=np.float32)
    wk = np.asarray(wk, dtype=np.float32)
    wv = np.asarray(wv, dtype=np.float32)
    wo = np.asarray(wo, dtype=np.float32)
    wi = np.asarray(wi, dtype=np.float32)
    wmo = np.asarray(wmo, dtype=np.float32)
    rel_emb = np.asarray(rel_emb, dtype=np.float32)
    mask = np.asarray(decoder_mask).reshape(L, L)

    tril = np.tril(np.ones((L, L), dtype=bool))
    if np.array_equal(mask, tril):
        causal = True
    elif mask.all():
        causal = False
    else:
        raise NotImplementedError("only causal or all-true masks supported")

    in_maps = []
    for c in range(NCORES):
        b, g = divmod(c, 2)
        heads = np.arange(HC * g, HC * (g + 1))
        band = _make_band(rel_emb, heads, causal)        # [128, W, HC]
        band = np.ascontiguousarray(band.transpose(0, 2, 1)).astype(
            ml_dtypes.bfloat16)
        bfut = np.broadcast_to(
            (rel_emb[0][heads] - rel_emb[NUM_BUCKETS - 1][heads])
            .astype(np.float32), (128, HC)).copy()

        xT = inputs[b].T                                  # [E, L]
        x8 = _f8(xT)
        xr = _f8(16.0 * (xT - x8.astype(np.float32)))
        x8 = x8.reshape(ET, 128, L).transpose(1, 0, 2)    # [128, ET, L]
        xr = xr.reshape(ET, 128, L).transpose(1, 0, 2)

        wq_c = wq[:, heads, :]
        wk_c = wk[:, heads, :]
        wqs = np.stack(_split16(_stat_qk(wq_c), 16.0))    # [3,4,TB,128,2,128]
        wks = np.stack(_split16(_stat_qk(wk_c), 16.0))

        wv_c = wv[:, heads, :].reshape(E, HC * D)
        wvm = np.empty((3, 2, 128, TB, 2, 256), E4)
        for vh in range(2):
            N = wv_c[:, 256 * vh:256 * vh + 256]
            N = N.reshape(TB, 2, 128, 256).transpose(2, 0, 1, 3)
            s1, s2, s3 = _split16(N, 16.0)
            wvm[0, vh], wvm[1, vh], wvm[2, vh] = s1, s2, s3

        wi_c = wi[:, FC * g:FC * (g + 1)]
        wis = np.empty((3, FT, 128, TB, 2, 128), E4)
        for ft in range(FT):
            M = wi_c[:, 128 * ft:128 * ft + 128]
            M = M.reshape(TB, 2, 128, 128).transpose(2, 0, 1, 3)
            s1, s2, s3 = _split16(M, 16.0)
            wis[0, ft], wis[1, ft], wis[2, ft] = s1, s2, s3

        wmo_c = wmo[FC * g:FC * (g + 1), :]               # [FC, E]
        wm = wmo_c.reshape(FT, 128, E).transpose(1, 0, 2)  # [128, FT, E]
        m1 = _f8(32.0 * wm)
        m2 = _f8(32.0 * wm - m1.astype(np.float32))
        wmm = np.stack([m1, m2])

        wo_c = wo[heads]                                   # [HC, D, E]
        wos = wo_c.reshape(4, 2, 64, E).transpose(0, 1, 2, 3) \
            .reshape(4, 128, E).transpose(1, 0, 2)         # [128, 4, E]
        wos = np.ascontiguousarray(wos).astype(ml_dtypes.bfloat16)

        in_maps.append(dict(
            x8=np.ascontiguousarray(x8), xr=np.ascontiguousarray(xr),
            wqs=wqs, wks=wks, wvm=wvm, wis=wis, wmm=wmm,
            wos=wos, band=band,
            ident=np.eye(128, dtype=np.float32).astype(ml_dtypes.bfloat16),
            bfut=bfut,
        ))
    return in_maps, causal, inputs


def run(trace=False, **kw):
    in_maps, causal, inputs = _prep_in_maps(**kw)
    nc = _get_nc(causal)
    res = run_bass_kernel_spmd(nc, in_maps, list(range(NCORES)), trace=trace)
    out = np.empty((B, L, E), dtype=np.float32)
    for b in range(B):
        out[b] = (inputs[b]
                  + res.results[2 * b]["attn_out"]
                  + res.results[2 * b]["mlp_out"]
                  + res.results[2 * b + 1]["attn_out"]
                  + res.results[2 * b + 1]["mlp_out"])
    return out, res


def kernel(**inputs):
    out, _ = run(**inputs)
    return out
